# revision 18
# baseline (speedup 1.0000x reference)
"""Additive (Bahdanau) attention on 8 Trainium2 NeuronCores.

Reference computation (B=32, S=4096, H=512):
    q_t    = query @ Wq + bq                      # [B,1,H]
    v_t    = values @ Wv + bv                     # [B,S,H]
    scores = tanh(q_t + v_t) @ wc + bc            # [B,S]
    attn   = softmax(scores)                      # [B,S]  (mask is all-ones)
    context= attn @ values                        # [B,H]
    returns (context, attn)

Sharding: data-parallel over batch, 4 batches per core, weights replicated.

Device kernel design (per core, per batch b):
  * values[b] is DMA-loaded with an fp32->fp16 cast (SWDGE) into SBUF layout
    vbf[p, a, h] = values[32p+a, h]  (s = 32p + a, so each partition reads
    contiguous DRAM), one DMA per 512-column window so compute starts early.
  * PE transposes (matmul vs identity, fp16-in-PSUM staging) build vT tiles;
    the big matmul v_t = values @ Wv runs in fp16 with fp32 PSUM accumulation.
  * ACT applies tanh with per-partition bias qb = q_t[b] + bq + bv (the tiny
    q_t = query @ Wq is precomputed on host).
  * scores: DVE pre-weights tanh by wc (per-partition tensor_scalar) and
    tree-adds the four k-chunks, so the PE only does one ones-reduction
    matmul per 128 scores; results land directly in [p, a] layout in PSUM.
  * softmax skips max-subtraction (|scores| <= sum|wc| ~ 12, safe in fp32
    exp); bc is dropped entirely (softmax(x + c) == softmax(x)).
  * context = attn @ values reuses the natural-layout fp16 values tiles.
"""

import os
import sys

import numpy as np

for _p in ("/opt/trn_rl_repo", "/root/.axon_site/_ro/trn_rl_repo"):
    if os.path.isdir(_p) and _p not in sys.path:
        sys.path.append(_p)

from contextlib import ExitStack

import concourse.bass as bass  # noqa: E402
import concourse.tile as tile  # noqa: E402
from concourse import bacc, mybir  # noqa: E402
from concourse.bass_utils import run_bass_kernel_spmd  # noqa: E402
from concourse.masks import make_identity  # noqa: E402

B, S, H = 32, 4096, 512
NCORES = 8
BPC = B // NCORES  # batches per core
P = 128            # SBUF partitions
AB = S // P        # 32 a-blocks; s = 32*p + a
HC = H // P        # 4 h-chunks (also k-chunks)
QW = 4             # a-blocks per window
NW = AB // QW      # 8 windows per batch

F32 = mybir.dt.float32
F16 = mybir.dt.float16
AF = mybir.ActivationFunctionType

_CACHE = {}


def _emit(ctx, tc, values_d, qbt_d, wv_d, wc_d, ctx_d, attn_d):
    nc = tc.nc

    const = ctx.enter_context(tc.tile_pool(name="const", bufs=1))
    vpool = ctx.enter_context(tc.tile_pool(name="vbf", bufs=12))
    vtpool = ctx.enter_context(tc.tile_pool(name="vt", bufs=4))
    thpool = ctx.enter_context(tc.tile_pool(name="th", bufs=8))
    spool = ctx.enter_context(tc.tile_pool(name="small", bufs=2))

    tp_ps = ctx.enter_context(tc.tile_pool(name="tp_ps", bufs=3, space="PSUM"))
    vt_ps = ctx.enter_context(tc.tile_pool(name="vt_ps", bufs=2, space="PSUM"))
    m_ps = ctx.enter_context(tc.tile_pool(name="m_ps", bufs=1, space="PSUM"))

    # Identity first (gpsimd queue) so transposes aren't blocked on consts.
    ident = const.tile([P, P], F16)
    make_identity(nc, ident)

    def load_window(vsrc, w):
        vw = vpool.tile([P, QW, H], F16)
        nc.gpsimd.dma_start(out=vw, in_=vsrc[:, w * QW:(w + 1) * QW, :])
        return vw

    # DMA queue order matters at startup: stage window 0 in halves and Wv in
    # h-chunks so the first transposes and matmuls unblock as early as
    # possible instead of queueing behind whole-tensor loads.
    vsrc0 = values_d[0].rearrange("(p a) h -> p a h", p=P)
    w0 = vpool.tile([P, QW, H], F16)
    nc.gpsimd.dma_start(out=w0[:, 0:2, :], in_=vsrc0[:, 0:2, :])

    wv_sb = const.tile([P, HC, H], F16)  # [h-part, hc, k], cast during DMA
    wv_src = wv_d.rearrange("(c p) k -> p c k", p=P)
    nc.gpsimd.dma_start(out=wv_sb[:, 0, :], in_=wv_src[:, 0, :])

    nc.gpsimd.dma_start(out=w0[:, 2:QW, :], in_=vsrc0[:, 2:QW, :])
    nc.gpsimd.dma_start(out=wv_sb[:, 1, :], in_=wv_src[:, 1, :])

    # wc in [k, c] layout, fp16 (matmul rhs for the scores reduction).
    wc_sb = const.tile([P, HC], F16)
    nc.gpsimd.dma_start(out=wc_sb, in_=wc_d.rearrange("(c p) -> p c", p=P))
    # qb^T in [k, c, b] layout, fp32 (bias for tanh).
    qbt_sb = const.tile([P, HC, BPC], F32)
    nc.gpsimd.dma_start(out=qbt_sb, in_=qbt_d.rearrange("(c p) b -> p c b", p=P))

    vbf_first = [w0, load_window(vsrc0, 1)]
    nc.gpsimd.dma_start(out=wv_sb[:, 2, :], in_=wv_src[:, 2, :])
    nc.gpsimd.dma_start(out=wv_sb[:, 3, :], in_=wv_src[:, 3, :])

    for w in range(2, NW):
        vbf_first.append(load_window(vsrc0, w))

    ones_col = const.tile([P, 1], F32)
    nc.vector.memset(ones_col, 1.0)
    ones_row = const.tile([1, P], F32)
    nc.vector.memset(ones_row, 1.0)
    negshift = const.tile([P, 1], F32)
    nc.vector.memset(negshift, -3.0)

    ppool = ctx.enter_context(tc.tile_pool(name="parts", bufs=9))

    for b in range(BPC):
        if b == 0:
            vbf_w = vbf_first
        else:
            vsrc = values_d[b].rearrange("(p a) h -> p a h", p=P)
            vbf_w = [load_window(vsrc, w) for w in range(NW)]

        # esc16[p, a] = exp(scores[s = 32p + a] - 3); |scores| <= sum|wc| ~ 12
        # so exp(scores - 3) <= e^9 stays in fp16 range with good resolution.
        esc16 = spool.tile([P, AB], F16, tag="esc")
        parts = []
        # unnormalized context accumulator: cps = sum_s exp(scores[s]-3) * v[s]
        cps = m_ps.tile([1, H], F32, tag="ctx")
        th_w = [None] * NW

        def emit_scores(w):
            # scT[p, j] = scores[s = 32p + (w*QW+j)]: per column, accumulate
            # the 4 k-chunks with tanh tiles stationary; then exp on ACT.
            sc = m_ps.tile([P, QW], F32, tag="sc")
            for j in range(QW):
                for kc in range(HC):
                    nc.tensor.matmul(
                        sc[:, j: j + 1],
                        lhsT=th_w[w][kc][:, j * P:(j + 1) * P],
                        rhs=wc_sb[:, kc: kc + 1],
                        start=(kc == 0),
                        stop=(kc == HC - 1),
                        skip_group_check=True,
                    )
            pw = ppool.tile([P, 1], F32)
            nc.scalar.activation(
                esc16[:, w * QW:(w + 1) * QW], sc, AF.Exp,
                bias=negshift, accum_out=pw,
            )
            parts.append(pw)

        def emit_ctx(w):
            for j in range(QW):
                a = w * QW + j
                nc.tensor.matmul(
                    cps,
                    lhsT=esc16[:, a: a + 1],
                    rhs=vbf_w[w][:, j, :],
                    start=(a == 0),
                    stop=(a == AB - 1),
                )

        for w in range(NW):
            # --- transpose: vt[half][h', hcl, j*128+p] = values[32p+a0+j, h]
            # (h = (2*half+hcl)*128 + h'). fp16 PSUM staging: one bank holds
            # two h-chunks; the DVE copy out runs in 2x mode.
            vt = []
            for half in range(HC // 2):
                tp = tp_ps.tile([P, 2, QW * P], F16)
                for hcl in range(2):
                    hc = 2 * half + hcl
                    for j in range(QW):
                        nc.tensor.matmul(
                            tp[:, hcl, j * P:(j + 1) * P],
                            lhsT=vbf_w[w][:, j, hc * P:(hc + 1) * P],
                            rhs=ident,
                            is_transpose=True,
                            start=True,
                            stop=True,
                        )
                v = vtpool.tile([P, 2, QW * P], F16)
                nc.vector.tensor_copy(v, tp)
                vt.append(v)

            # --- v_t matmul + tanh, per k-chunk
            ths = []
            for kc in range(HC):
                vp = vt_ps.tile([P, QW * P], F32)
                for hc in range(HC):
                    nc.tensor.matmul(
                        vp,
                        lhsT=wv_sb[:, hc, kc * P:(kc + 1) * P],
                        rhs=vt[hc // 2][:, hc % 2, :],
                        start=(hc == 0),
                        stop=(hc == HC - 1),
                    )
                t = thpool.tile([P, QW * P], F16)
                nc.scalar.activation(
                    t, vp, AF.Tanh,
                    bias=qbt_sb[:, kc, b: b + 1],
                )
                ths.append(t)
            th_w[w] = ths

            # software pipeline on PE: scores one window behind, ctx two behind
            if w >= 1:
                emit_scores(w - 1)
            if w >= 2:
                emit_ctx(w - 2)

        emit_scores(NW - 1)
        emit_ctx(NW - 2)
        emit_ctx(NW - 1)

        # --- softmax normalization
        p01 = ppool.tile([P, 1], F32)
        nc.vector.tensor_add(p01, parts[0], parts[1])
        p23 = ppool.tile([P, 1], F32)
        nc.vector.tensor_add(p23, parts[2], parts[3])
        p45 = ppool.tile([P, 1], F32)
        nc.vector.tensor_add(p45, parts[4], parts[5])
        p67 = ppool.tile([P, 1], F32)
        nc.vector.tensor_add(p67, parts[6], parts[7])
        q0 = ppool.tile([P, 1], F32)
        nc.vector.tensor_add(q0, p01, p23)
        q1 = ppool.tile([P, 1], F32)
        nc.vector.tensor_add(q1, p45, p67)
        part = ppool.tile([P, 1], F32)
        nc.vector.tensor_add(part, q0, q1)

        tot = m_ps.tile([1, 1], F32, tag="m")
        nc.tensor.matmul(tot, lhsT=part, rhs=ones_col, start=True, stop=True)
        recip = spool.tile([1, 1], F32)
        nc.vector.reciprocal(recip, tot)
        rb_p = m_ps.tile([P, 1], F32, tag="m")
        nc.tensor.matmul(rb_p, lhsT=ones_row, rhs=recip, start=True, stop=True)
        rb = spool.tile([P, 1], F32)
        nc.vector.tensor_copy(rb, rb_p)

        attn_sb = spool.tile([P, AB], F32)
        nc.vector.tensor_scalar_mul(attn_sb, esc16, rb)
        nc.sync.dma_start(
            out=attn_d[b].rearrange("(p a) -> p a", p=P), in_=attn_sb
        )

        ctx_sb = spool.tile([1, H], F32)
        nc.vector.tensor_scalar_mul(ctx_sb, cps, recip)
        nc.sync.dma_start(out=ctx_d[b], in_=ctx_sb)


def _build_nc():
    nc = bacc.Bacc("TRN2", target_bir_lowering=False, debug=False)
    values_d = nc.dram_tensor("values", [BPC, S, H], F32, kind="ExternalInput").ap()
    qbt_d = nc.dram_tensor("qbt", [H, BPC], F32, kind="ExternalInput").ap()
    wv_d = nc.dram_tensor("wv", [H, H], F32, kind="ExternalInput").ap()
    wc_d = nc.dram_tensor("wc", [H], F32, kind="ExternalInput").ap()
    ctx_d = nc.dram_tensor("context_out", [BPC, H], F32, kind="ExternalOutput").ap()
    attn_d = nc.dram_tensor("attn_out", [BPC, S], F32, kind="ExternalOutput").ap()

    with tile.TileContext(nc) as tc:
        with ExitStack() as ctx:
            _emit(ctx, tc, values_d, qbt_d, wv_d, wc_d, ctx_d, attn_d)
    nc.compile()
    return nc


def _get_nc():
    if "nc" not in _CACHE:
        _CACHE["nc"] = _build_nc()
    return _CACHE["nc"]


def _numpy_fallback(query, values, mask, Wq, bq, Wv, bv, wc, bc):
    q_t = query @ Wq + bq                       # [B,1,H]
    v_t = values @ Wv + bv                      # [B,S,H]
    t = np.tanh(q_t + v_t)
    scores = t @ wc + bc[0]                     # [B,S]
    scores = np.where(mask == 0, -np.inf, scores)
    m = scores.max(axis=1, keepdims=True)
    e = np.exp(scores - m)
    attn = e / e.sum(axis=1, keepdims=True)
    context = np.einsum("bs,bsh->bh", attn, values)
    return context.astype(np.float32), attn.astype(np.float32)


def kernel(query, values, mask, Wq, bq, Wv, bv, wc, bc, _spmd_kwargs=None):
    query = np.asarray(query, dtype=np.float32)
    values = np.ascontiguousarray(np.asarray(values, dtype=np.float32))
    mask = np.asarray(mask)
    Wq = np.asarray(Wq, dtype=np.float32)
    bq = np.asarray(bq, dtype=np.float32)
    Wv = np.ascontiguousarray(np.asarray(Wv, dtype=np.float32))
    bv = np.asarray(bv, dtype=np.float32)
    wc = np.ascontiguousarray(np.asarray(wc, dtype=np.float32))
    bc = np.asarray(bc, dtype=np.float32)

    if not np.all(mask != 0):
        # The spec pins mask to all-ones; keep a correct fallback anyway.
        return _numpy_fallback(query, values, mask, Wq, bq, Wv, bv, wc, bc)

    # Tiny host precompute: qb = query @ Wq + bq + bv  (16.8 MFLOP of 68.7 GFLOP)
    qb = query[:, 0, :] @ Wq + bq + bv          # [B, H]

    nc = _get_nc()
    in_maps = []
    for c in range(NCORES):
        sl = slice(c * BPC, (c + 1) * BPC)
        in_maps.append(
            {
                "values": values[sl],
                "qbt": np.ascontiguousarray(qb[sl].T),
                "wv": Wv,
                "wc": wc,
            }
        )
    res = run_bass_kernel_spmd(
        nc, in_maps, core_ids=list(range(NCORES)), **(_spmd_kwargs or {})
    )
    context = np.concatenate([r["context_out"] for r in res.results], axis=0)
    attn = np.concatenate([r["attn_out"] for r in res.results], axis=0)
    if _spmd_kwargs:
        _CACHE["last_results"] = res
    return context, attn


# revision 20
# speedup vs baseline: 1.0253x; 1.0253x over previous
"""Additive (Bahdanau) attention on 8 Trainium2 NeuronCores.

Reference computation (B=32, S=4096, H=512):
    q_t    = query @ Wq + bq                      # [B,1,H]
    v_t    = values @ Wv + bv                     # [B,S,H]
    scores = tanh(q_t + v_t) @ wc + bc            # [B,S]
    attn   = softmax(scores)                      # [B,S]  (mask is all-ones)
    context= attn @ values                        # [B,H]
    returns (context, attn)

Sharding: data-parallel over batch, 4 batches per core, weights replicated.

Device kernel design (per core, per batch b):
  * values[b] is DMA-loaded with an fp32->fp16 cast (SWDGE) into SBUF layout
    vbf[p, a, h] = values[32p+a, h]  (s = 32p + a, so each partition reads
    contiguous DRAM), one DMA per 512-column window so compute starts early.
    The first two windows and Wv go through HWDGE (sync engine) as fp32 with
    a DVE cast, because SWDGE descriptor emission serializes ~2us per DMA on
    the Q7 and would otherwise gate the pipeline start.
  * PE transposes (matmul vs identity, fp16-in-PSUM staging) build vT tiles;
    the big matmul v_t = values @ Wv runs in fp16 with fp32 PSUM accumulation.
  * ACT applies tanh with per-partition bias qb = q_t[b] + bq + bv (the tiny
    q_t = query @ Wq is precomputed on host).
  * scores: DVE pre-weights tanh by wc (per-partition tensor_scalar) and
    tree-adds the four k-chunks; the PE does one ones-reduction matmul per
    128-score column, landing scores directly in [p, a] layout.
  * softmax skips max-subtraction; exp runs per window with a constant -3
    bias so exp(scores-3) provably fits fp16 (|scores| <= sum|wc| ~ 12, so
    exp <= e^9 < 65504); context accumulates exp-weighted values pipelined
    two windows behind, and both outputs are normalized by 1/sum at the end.
    bc is dropped entirely (softmax(x + c) == softmax(x)).
"""

import os
import sys

import numpy as np

for _p in ("/opt/trn_rl_repo", "/root/.axon_site/_ro/trn_rl_repo"):
    if os.path.isdir(_p) and _p not in sys.path:
        sys.path.append(_p)

from contextlib import ExitStack

import concourse.bass as bass  # noqa: E402
import concourse.tile as tile  # noqa: E402
from concourse import bacc, mybir  # noqa: E402
from concourse.bass_utils import run_bass_kernel_spmd  # noqa: E402
from concourse.masks import make_identity  # noqa: E402

B, S, H = 32, 4096, 512
NCORES = 8
BPC = B // NCORES  # batches per core
P = 128            # SBUF partitions
AB = S // P        # 32 a-blocks; s = 32*p + a
HC = H // P        # 4 h-chunks (also k-chunks)
QW = 4             # a-blocks per window
NW = AB // QW      # 8 windows per batch

F32 = mybir.dt.float32
F16 = mybir.dt.float16
AF = mybir.ActivationFunctionType

_CACHE = {}


def _emit(ctx, tc, values_d, qbt_d, wv_d, wc_d, ctx_d, attn_d):
    nc = tc.nc

    const = ctx.enter_context(tc.tile_pool(name="const", bufs=1))
    vpool = ctx.enter_context(tc.tile_pool(name="vbf", bufs=10))
    v32pool = ctx.enter_context(tc.tile_pool(name="v32", bufs=1))
    vtpool = ctx.enter_context(tc.tile_pool(name="vt", bufs=4))
    thpool = ctx.enter_context(tc.tile_pool(name="th", bufs=8))
    wpool = ctx.enter_context(tc.tile_pool(name="wsc", bufs=4))
    spool = ctx.enter_context(tc.tile_pool(name="small", bufs=2))
    ppool = ctx.enter_context(tc.tile_pool(name="parts", bufs=9))

    tp_ps = ctx.enter_context(tc.tile_pool(name="tp_ps", bufs=3, space="PSUM"))
    vt_ps = ctx.enter_context(tc.tile_pool(name="vt_ps", bufs=2, space="PSUM"))
    m_ps = ctx.enter_context(tc.tile_pool(name="m_ps", bufs=1, space="PSUM"))

    # Identity first (gpsimd queue) so transposes aren't blocked on consts.
    ident = const.tile([P, P], F16)
    make_identity(nc, ident)

    def load_window(vsrc, w):
        vw = vpool.tile([P, QW, H], F16)
        nc.gpsimd.dma_start(out=vw, in_=vsrc[:, w * QW:(w + 1) * QW, :])
        return vw

    # --- startup staging on HWDGE (no Q7 serialization) ---
    vsrc0 = values_d[0].rearrange("(p a) h -> p a h", p=P)
    w0_32 = v32pool.tile([P, QW, H], F32)
    nc.sync.dma_start(out=w0_32[:, 0:2, :], in_=vsrc0[:, 0:2, :])
    wv_32 = v32pool.tile([P, HC, H], F32, tag="wv32")
    nc.sync.dma_start(out=wv_32, in_=wv_d.rearrange("(c p) k -> p c k", p=P))
    nc.sync.dma_start(out=w0_32[:, 2:QW, :], in_=vsrc0[:, 2:QW, :])
    w1_32 = v32pool.tile([P, QW, H], F32)
    nc.sync.dma_start(out=w1_32, in_=vsrc0[:, QW:2 * QW, :])

    w0 = vpool.tile([P, QW, H], F16)
    nc.vector.tensor_copy(w0[:, 0:2, :], w0_32[:, 0:2, :])
    wv_sb = const.tile([P, HC, H], F16)
    nc.vector.tensor_copy(wv_sb[:, 0:2, :], wv_32[:, 0:2, :])
    nc.vector.tensor_copy(w0[:, 2:QW, :], w0_32[:, 2:QW, :])
    nc.vector.tensor_copy(wv_sb[:, 2:HC, :], wv_32[:, 2:HC, :])
    w1 = vpool.tile([P, QW, H], F16)
    nc.vector.tensor_copy(w1, w1_32)

    # --- small consts + remaining windows on the gpsimd (SWDGE cast) queue ---
    # wc in [k, c] layout, fp32 (per-partition tensor_scalar operand).
    wc_sb = const.tile([P, HC], F32)
    nc.gpsimd.dma_start(out=wc_sb, in_=wc_d.rearrange("(c p) -> p c", p=P))
    # qb^T in [k, c, b] layout, fp32 (bias for tanh).
    qbt_sb = const.tile([P, HC, BPC], F32)
    nc.gpsimd.dma_start(out=qbt_sb, in_=qbt_d.rearrange("(c p) b -> p c b", p=P))

    vbf_first = [w0, w1]
    for w in range(2, NW):
        vbf_first.append(load_window(vsrc0, w))

    ones_col = const.tile([P, 1], F32)
    nc.vector.memset(ones_col, 1.0)
    ones_row = const.tile([1, P], F32)
    nc.vector.memset(ones_row, 1.0)
    ones_f16 = const.tile([P, 1], F16)
    nc.vector.memset(ones_f16, 1.0)
    negshift = const.tile([P, 1], F32)
    nc.vector.memset(negshift, -3.0)

    for b in range(BPC):
        if b == 0:
            vbf_w = vbf_first
        else:
            vsrc = values_d[b].rearrange("(p a) h -> p a h", p=P)
            vbf_w = [load_window(vsrc, w) for w in range(NW)]

        # esc16[p, a] = exp(scores[s = 32p + a] - 3); |scores| <= sum|wc| ~ 12
        # so exp(scores - 3) <= e^9 stays in fp16 range with good resolution.
        esc16 = spool.tile([P, AB], F16, tag="esc")
        parts = []
        # unnormalized context accumulator: cps = sum_s exp(scores[s]-3) * v[s]
        cps = m_ps.tile([1, H], F32, tag="ctx")
        sum4_w = [None] * NW

        def emit_scores(w):
            # one ones-reduction matmul per 128-score column, then exp on ACT
            sc = m_ps.tile([P, QW], F32, tag="sc")
            for j in range(QW):
                nc.tensor.matmul(
                    sc[:, j: j + 1],
                    lhsT=sum4_w[w][:, j * P:(j + 1) * P],
                    rhs=ones_f16,
                    start=True,
                    stop=True,
                    skip_group_check=True,
                )
            pw = ppool.tile([P, 1], F32)
            nc.scalar.activation(
                esc16[:, w * QW:(w + 1) * QW], sc, AF.Exp,
                bias=negshift, accum_out=pw,
            )
            parts.append(pw)

        def emit_ctx(w):
            for j in range(QW):
                a = w * QW + j
                nc.tensor.matmul(
                    cps,
                    lhsT=esc16[:, a: a + 1],
                    rhs=vbf_w[w][:, j, :],
                    start=(a == 0),
                    stop=(a == AB - 1),
                )

        for w in range(NW):
            # --- transpose: vt[half][h', hcl, j*128+p] = values[32p+a0+j, h]
            # (h = (2*half+hcl)*128 + h'). fp16 PSUM staging: one bank holds
            # two h-chunks; the DVE copy out runs in 2x mode.
            vt = []
            for half in range(HC // 2):
                tp = tp_ps.tile([P, 2, QW * P], F16)
                for hcl in range(2):
                    hc = 2 * half + hcl
                    for j in range(QW):
                        nc.tensor.matmul(
                            tp[:, hcl, j * P:(j + 1) * P],
                            lhsT=vbf_w[w][:, j, hc * P:(hc + 1) * P],
                            rhs=ident,
                            is_transpose=True,
                            start=True,
                            stop=True,
                        )
                v = vtpool.tile([P, 2, QW * P], F16)
                nc.vector.tensor_copy(v, tp)
                vt.append(v)

            # --- v_t matmul + tanh, per k-chunk; DVE pre-weights by wc.
            wsc = []
            for kc in range(HC):
                vp = vt_ps.tile([P, QW * P], F32)
                for hc in range(HC):
                    nc.tensor.matmul(
                        vp,
                        lhsT=wv_sb[:, hc, kc * P:(kc + 1) * P],
                        rhs=vt[hc // 2][:, hc % 2, :],
                        start=(hc == 0),
                        stop=(hc == HC - 1),
                    )
                t = thpool.tile([P, QW * P], F16)
                nc.scalar.activation(
                    t, vp, AF.Tanh,
                    bias=qbt_sb[:, kc, b: b + 1],
                )
                ws = wpool.tile([P, QW * P], F16, tag="w")
                nc.vector.tensor_scalar_mul(ws, t, wc_sb[:, kc: kc + 1])
                wsc.append(ws)

            # tree-add the 4 k-chunks on DVE
            s01 = wpool.tile([P, QW * P], F16, tag="s01")
            nc.vector.tensor_add(s01, wsc[0], wsc[1])
            s23 = wpool.tile([P, QW * P], F16, tag="s23")
            nc.vector.tensor_add(s23, wsc[2], wsc[3])
            sum4 = wpool.tile([P, QW * P], F16, tag="sum4")
            nc.vector.tensor_add(sum4, s01, s23)
            sum4_w[w] = sum4

            # software pipeline on PE: scores one window behind, ctx two behind
            if w >= 1:
                emit_scores(w - 1)
            if w >= 2:
                emit_ctx(w - 2)

        emit_scores(NW - 1)
        emit_ctx(NW - 2)
        emit_ctx(NW - 1)

        # --- softmax normalization
        p01 = ppool.tile([P, 1], F32)
        nc.vector.tensor_add(p01, parts[0], parts[1])
        p23 = ppool.tile([P, 1], F32)
        nc.vector.tensor_add(p23, parts[2], parts[3])
        p45 = ppool.tile([P, 1], F32)
        nc.vector.tensor_add(p45, parts[4], parts[5])
        p67 = ppool.tile([P, 1], F32)
        nc.vector.tensor_add(p67, parts[6], parts[7])
        q0 = ppool.tile([P, 1], F32)
        nc.vector.tensor_add(q0, p01, p23)
        q1 = ppool.tile([P, 1], F32)
        nc.vector.tensor_add(q1, p45, p67)
        part = ppool.tile([P, 1], F32)
        nc.vector.tensor_add(part, q0, q1)

        tot = m_ps.tile([1, 1], F32, tag="m")
        nc.tensor.matmul(tot, lhsT=part, rhs=ones_col, start=True, stop=True)
        recip = spool.tile([1, 1], F32)
        nc.vector.reciprocal(recip, tot)
        rb_p = m_ps.tile([P, 1], F32, tag="m")
        nc.tensor.matmul(rb_p, lhsT=ones_row, rhs=recip, start=True, stop=True)
        rb = spool.tile([P, 1], F32)
        nc.vector.tensor_copy(rb, rb_p)

        attn_sb = spool.tile([P, AB], F32)
        nc.vector.tensor_scalar_mul(attn_sb, esc16, rb)
        nc.sync.dma_start(
            out=attn_d[b].rearrange("(p a) -> p a", p=P), in_=attn_sb
        )

        ctx_sb = spool.tile([1, H], F32)
        nc.vector.tensor_scalar_mul(ctx_sb, cps, recip)
        nc.sync.dma_start(out=ctx_d[b], in_=ctx_sb)


def _build_nc():
    nc = bacc.Bacc("TRN2", target_bir_lowering=False, debug=False)
    values_d = nc.dram_tensor("values", [BPC, S, H], F32, kind="ExternalInput").ap()
    qbt_d = nc.dram_tensor("qbt", [H, BPC], F32, kind="ExternalInput").ap()
    wv_d = nc.dram_tensor("wv", [H, H], F32, kind="ExternalInput").ap()
    wc_d = nc.dram_tensor("wc", [H], F32, kind="ExternalInput").ap()
    ctx_d = nc.dram_tensor("context_out", [BPC, H], F32, kind="ExternalOutput").ap()
    attn_d = nc.dram_tensor("attn_out", [BPC, S], F32, kind="ExternalOutput").ap()

    with tile.TileContext(nc) as tc:
        with ExitStack() as ctx:
            _emit(ctx, tc, values_d, qbt_d, wv_d, wc_d, ctx_d, attn_d)
    nc.compile()
    return nc


def _get_nc():
    if "nc" not in _CACHE:
        _CACHE["nc"] = _build_nc()
    return _CACHE["nc"]


def _numpy_fallback(query, values, mask, Wq, bq, Wv, bv, wc, bc):
    q_t = query @ Wq + bq                       # [B,1,H]
    v_t = values @ Wv + bv                      # [B,S,H]
    t = np.tanh(q_t + v_t)
    scores = t @ wc + bc[0]                     # [B,S]
    scores = np.where(mask == 0, -np.inf, scores)
    m = scores.max(axis=1, keepdims=True)
    e = np.exp(scores - m)
    attn = e / e.sum(axis=1, keepdims=True)
    context = np.einsum("bs,bsh->bh", attn, values)
    return context.astype(np.float32), attn.astype(np.float32)


def kernel(query, values, mask, Wq, bq, Wv, bv, wc, bc, _spmd_kwargs=None):
    query = np.asarray(query, dtype=np.float32)
    values = np.ascontiguousarray(np.asarray(values, dtype=np.float32))
    mask = np.asarray(mask)
    Wq = np.asarray(Wq, dtype=np.float32)
    bq = np.asarray(bq, dtype=np.float32)
    Wv = np.ascontiguousarray(np.asarray(Wv, dtype=np.float32))
    bv = np.asarray(bv, dtype=np.float32)
    wc = np.ascontiguousarray(np.asarray(wc, dtype=np.float32))
    bc = np.asarray(bc, dtype=np.float32)

    if not np.all(mask != 0):
        # The spec pins mask to all-ones; keep a correct fallback anyway.
        return _numpy_fallback(query, values, mask, Wq, bq, Wv, bv, wc, bc)

    # Tiny host precompute: qb = query @ Wq + bq + bv  (16.8 MFLOP of 68.7 GFLOP)
    qb = query[:, 0, :] @ Wq + bq + bv          # [B, H]

    nc = _get_nc()
    in_maps = []
    for c in range(NCORES):
        sl = slice(c * BPC, (c + 1) * BPC)
        in_maps.append(
            {
                "values": values[sl],
                "qbt": np.ascontiguousarray(qb[sl].T),
                "wv": Wv,
                "wc": wc,
            }
        )
    res = run_bass_kernel_spmd(
        nc, in_maps, core_ids=list(range(NCORES)), **(_spmd_kwargs or {})
    )
    context = np.concatenate([r["context_out"] for r in res.results], axis=0)
    attn = np.concatenate([r["attn_out"] for r in res.results], axis=0)
    if _spmd_kwargs:
        _CACHE["last_results"] = res
    return context, attn
